# revision 14
# baseline (speedup 1.0000x reference)
"""Trainium2 Bass kernel for CAML-style sparse attention (nn_CAML_39977555591512).

Model (per batch element b):
    x   = embed[text[b]]                      # (L, E)
    Z   = tanh(conv1d(x, conv_w) + conv_b)    # (T, F), T = L + 1
    S   = Z @ Q_w.T                           # (T, C) scores (transposed layout)
    a   = softmax(S, axis=T)                  # alpha^T
    E   = a^T @ Z                             # (C, F)
    y   = sum_f out_w * E + out_b             # (C,)
Returns (logits (B, C), alpha (B, C, T)).

Sharding: pure data-parallel over batch (B = 8 = n_cores); every core runs the
identical single-core program on its own batch row.  All device work is done in
the [t-partition, c-free] orientation; alpha is written transposed (plus helper
padding) and the host restores the reference layout while gathering.
"""

import os
import numpy as np
from contextlib import ExitStack

os.environ.setdefault("MYCRO_LOCAL_CACHE", "1")

import concourse.bass as bass
import concourse.tile as tile
from concourse import bacc, mybir
from concourse.bass_utils import run_bass_kernel_spmd
from concourse.masks import make_identity

F32 = mybir.dt.float32
BF16 = mybir.dt.bfloat16
I32 = mybir.dt.int32

N_CORES = 8


def _cfg(B=8, L=2500, V=50000, E=100, F=50, K=10, C=8921):
    PAD = K // 2
    T_OUT = L + 2 * PAD - K + 1          # conv output length (2501)
    TI = (T_OUT + 127) // 128            # t-tiles of 128 (20)
    T2 = TI * 128                        # padded length (2560)
    T_TAIL = T_OUT - (TI - 1) * 128      # valid rows in last t-tile (69)
    CJ = (C + 127) // 128                # 128-wide class tiles (70)
    C2 = CJ * 128                        # padded classes (8960)
    XW = T2 + K - 1                      # padded x columns (2569)
    chunks = []
    c0 = 0
    while c0 < C2:
        chunks.append((c0, min(512, C2 - c0)))
        c0 += 512
    return dict(B=B, L=L, V=V, E=E, F=F, K=K, C=C, PAD=PAD, T_OUT=T_OUT, TI=TI,
                T2=T2, T_TAIL=T_TAIL, CJ=CJ, C2=C2, XW=XW, chunks=chunks)


def build_kernel(cfg=None, reps=1):
    """Build + compile the single-core Bacc program (run SPMD on 8 cores)."""
    cfg = cfg or _cfg()
    V, E, F, K = cfg["V"], cfg["E"], cfg["F"], cfg["K"]
    TI, T2, T_TAIL = cfg["TI"], cfg["T2"], cfg["T_TAIL"]
    CJ, C2, XW = cfg["CJ"], cfg["C2"], cfg["XW"]
    chunks = cfg["chunks"]

    nc = bacc.Bacc("TRN2", target_bir_lowering=False, debug=False,
                   num_devices=N_CORES)

    # ---- DRAM I/O (host pre-packs layouts; see _prep_inputs) ----
    text_ap = nc.dram_tensor("text", [128, TI], I32, kind="ExternalInput").ap()
    embed_ap = nc.dram_tensor("embed", [V, E], F32, kind="ExternalInput").ap()
    convwT_ap = nc.dram_tensor("convwT", [E, K * F], F32, kind="ExternalInput").ap()
    convb_ap = nc.dram_tensor("convb", [F, 1], F32, kind="ExternalInput").ap()
    qwT_ap = nc.dram_tensor("qwT", [F, C2], F32, kind="ExternalInput").ap()
    outw_ap = nc.dram_tensor("outwc", [128, CJ * F], F32, kind="ExternalInput").ap()
    outb_ap = nc.dram_tensor("outbc", [128, CJ], F32, kind="ExternalInput").ap()
    alphaT_ap = nc.dram_tensor("alphaT", [128, TI, cfg["C"]], F32,
                               kind="ExternalOutput").ap()
    logits_ap = nc.dram_tensor("logits", [128, CJ], F32, kind="ExternalOutput").ap()

    with tile.TileContext(nc) as tc, ExitStack() as ctx:
        consts = ctx.enter_context(tc.tile_pool(name="consts", bufs=1))
        persist = ctx.enter_context(tc.tile_pool(name="persist", bufs=1))
        work = ctx.enter_context(tc.tile_pool(name="work", bufs=3))
        exps_pool = ctx.enter_context(tc.tile_pool(name="exps", bufs=2))
        alpha_pool = ctx.enter_context(tc.tile_pool(name="alpha", bufs=2))
        ps_scores = ctx.enter_context(tc.tile_pool(name="ps_scores", bufs=2, space="PSUM"))
        ps_ehat = ctx.enter_context(tc.tile_pool(name="ps_ehat", bufs=2, space="PSUM"))
        ps_misc = ctx.enter_context(tc.tile_pool(name="ps_misc", bufs=2, space="PSUM"))

        # ---- constants ----
        identity = consts.tile([128, 128], F32)
        make_identity(nc, identity[:])
        ones_row = consts.tile([1, 128], F32)
        nc.vector.memset(ones_row[:], 1.0)

        txt = consts.tile([128, TI], I32)
        nc.sync.dma_start(txt[:], text_ap[:])
        convwT = consts.tile([E, K * F], F32)
        nc.sync.dma_start(convwT[:], convwT_ap[:])
        convb = consts.tile([F, 1], F32)
        nc.sync.dma_start(convb[:], convb_ap[:])
        qwT = consts.tile([F, C2], F32)
        nc.sync.dma_start(qwT[:], qwT_ap[:])
        outw = consts.tile([128, CJ * F], F32)
        nc.sync.dma_start(outw[:], outw_ap[:])
        outb = consts.tile([128, CJ], F32)
        nc.sync.dma_start(outb[:], outb_ap[:])

        for _rep in range(reps):
            # ---- embedding gather -> xT [E, XW] (x_pad transposed) ----
            xT = persist.tile([E, XW], F32)
            nc.vector.memset(xT[:], 0.0)
            for g in range(TI):
                gt = work.tile([128, E], F32, tag="gather")
                nc.gpsimd.indirect_dma_start(
                    out=gt[:], out_offset=None, in_=embed_ap[:],
                    in_offset=bass.IndirectOffsetOnAxis(ap=txt[:, g:g + 1], axis=0),
                )
                xp = ps_misc.tile([E, 128], F32, tag="misc")
                nc.tensor.transpose(xp[:], gt[:], identity[:])
                nc.vector.tensor_copy(xT[:, cfg["PAD"] + 128 * g:
                                         cfg["PAD"] + 128 * g + 128], xp[:])

            # ---- conv1d + tanh -> ZT [F, T2] ----
            ZT = persist.tile([F, T2], F32)
            for t0 in range(0, T2, 512):
                w = min(512, T2 - t0)
                cp = ps_misc.tile([F, 512], F32, tag="misc")
                for k in range(K):
                    nc.tensor.matmul(cp[:, :w], lhsT=convwT[:, k * F:(k + 1) * F],
                                     rhs=xT[:, k + t0:k + t0 + w],
                                     start=(k == 0), stop=(k == K - 1))
                nc.scalar.activation(ZT[:, t0:t0 + w], cp[:, :w],
                                     mybir.ActivationFunctionType.Tanh,
                                     bias=convb[:], scale=1.0)

            # ---- Z_aug [128, 1+F] per t-tile (bf16; col 0 = ones; tail zeroed)
            # Layout puts the softmax-denominator row at PSUM partition 0
            # (DVE PSUM reads need a 32-aligned base partition).
            zaug = persist.tile([128, TI * (F + 1)], BF16)
            nc.vector.memset(zaug[:], 0.0)
            for ti in range(TI):
                zp = ps_misc.tile([128, F], F32, tag="misc")
                nc.tensor.transpose(zp[:], ZT[:, 128 * ti:128 * ti + 128],
                                    identity[:F, :F])
                rows = 128 if ti < TI - 1 else T_TAIL
                nc.vector.memset(zaug[:rows, ti * (F + 1):ti * (F + 1) + 1], 1.0)
                nc.vector.tensor_copy(zaug[:rows, ti * (F + 1) + 1:
                                           (ti + 1) * (F + 1)],
                                      zp[:rows, :])

            # ---- logits accumulators (class-partition layout) ----
            reds = persist.tile([128, CJ], F32, tag="reds")
            rcols = persist.tile([128, CJ], F32, tag="rcols")

            # ---- main loop over class chunks ----
            for (c0, NC) in chunks:
                ncr = min(NC, cfg["C"] - c0)  # columns actually stored
                exps = exps_pool.tile([128, TI * NC], BF16, tag="exps")
                # phase A: scores^T -> exp -> E_hat accumulation
                eh = ps_ehat.tile([F + 1, NC], F32, tag="ehat")
                for ti0 in range(0, TI, 2):
                    tis = [ti for ti in (ti0, ti0 + 1) if ti < TI]
                    sp = ps_scores.tile([128, 1024], F32, tag="scores")
                    for h, ti in enumerate(tis):
                        nc.tensor.matmul(sp[:, 512 * h:512 * h + NC],
                                         lhsT=ZT[:, 128 * ti:128 * ti + 128],
                                         rhs=qwT[:, c0:c0 + NC],
                                         start=True, stop=True)
                    if NC == 512 and len(tis) == 2:
                        nc.scalar.activation(
                            exps[:, ti0 * NC:(ti0 + 2) * NC],
                            sp[:], mybir.ActivationFunctionType.Exp)
                    else:
                        for h, ti in enumerate(tis):
                            nc.scalar.activation(
                                exps[:, ti * NC:(ti + 1) * NC],
                                sp[:, 512 * h:512 * h + NC],
                                mybir.ActivationFunctionType.Exp)
                    for ti in tis:
                        nc.tensor.matmul(
                            eh[:], lhsT=zaug[:, ti * (F + 1):(ti + 1) * (F + 1)],
                            rhs=exps[:, ti * NC:(ti + 1) * NC],
                            start=(ti == 0), stop=(ti == TI - 1))

                # phase B: normalize + store alpha, logits partials
                r_row = work.tile([1, NC], F32, tag="rrow")
                nc.vector.reciprocal(r_row[:], eh[0:1, :])
                rp = ps_misc.tile([128, NC], F32, tag="misc")
                nc.tensor.matmul(rp[:], lhsT=ones_row[:], rhs=r_row[:],
                                 start=True, stop=True)
                rb = work.tile([128, NC], BF16, tag="rb")
                nc.vector.tensor_copy(rb[:], rp[:])

                ehs = work.tile([F + 1, NC], F32, tag="ehs")
                nc.vector.tensor_copy(ehs[:], eh[:])

                alpha = alpha_pool.tile([128, TI * NC], BF16, tag="alpha")
                for ti in range(TI):
                    nc.vector.tensor_mul(alpha[:, ti * NC:(ti + 1) * NC],
                                         exps[:, ti * NC:(ti + 1) * NC], rb[:])
                nc.gpsimd.dma_start(
                    out=alphaT_ap[:, :, c0:c0 + ncr],
                    in_=alpha[:].rearrange("p (ti c) -> p ti c", ti=TI)[:, :, :ncr])

                for jj in range(NC // 128):
                    j = c0 // 128 + jj
                    et = ps_misc.tile([128, F + 1], F32, tag="misc")
                    nc.tensor.transpose(et[:], ehs[:, 128 * jj:128 * jj + 128],
                                        identity[:F + 1, :F + 1])
                    nc.vector.reciprocal(rcols[:, j:j + 1], et[:, 0:1])
                    pr = work.tile([128, F], F32, tag="prod")
                    nc.vector.tensor_mul(pr[:], et[:, 1:F + 1],
                                         outw[:, j * F:(j + 1) * F])
                    nc.vector.tensor_reduce(reds[:, j:j + 1], pr[:],
                                            axis=mybir.AxisListType.X,
                                            op=mybir.AluOpType.add)

            # ---- logits = reds * rcols + outb ----
            lg = persist.tile([128, CJ], F32, tag="lg")
            nc.vector.tensor_mul(lg[:], reds[:], rcols[:])
            nc.vector.tensor_add(lg[:], lg[:], outb[:])
            nc.sync.dma_start(logits_ap[:], lg[:])

    nc.compile()
    return nc, cfg


def _prep_inputs(cfg, text, embed_weight, conv_w, conv_b, Q_w, out_w, out_b):
    """Host-side packing of one batch row's inputs into the kernel layouts."""
    TI, T2, CJ, C2 = cfg["TI"], cfg["T2"], cfg["CJ"], cfg["C2"]
    E, F, K, C, L = cfg["E"], cfg["F"], cfg["K"], cfg["C"], cfg["L"]

    tp = np.zeros(T2, np.int32)
    tp[:L] = text
    txt2d = np.ascontiguousarray(tp.reshape(TI, 128).T)

    # slice k of convwT[:, k*F:(k+1)*F] must be conv_w[:, :, k].T = [E, F]
    convwT = np.ascontiguousarray(
        conv_w.transpose(1, 2, 0).reshape(E, K * F)).astype(np.float32)

    qwT = np.zeros((F, C2), np.float32)
    qwT[:, :C] = Q_w.T

    outw_pad = np.zeros((C2, F), np.float32)
    outw_pad[:C] = out_w
    outwc = np.ascontiguousarray(
        outw_pad.reshape(CJ, 128, F).transpose(1, 0, 2).reshape(128, CJ * F))

    outb_pad = np.zeros(C2, np.float32)
    outb_pad[:C] = out_b
    outbc = np.ascontiguousarray(outb_pad.reshape(CJ, 128).T)

    return {
        "text": txt2d,
        "embed": np.ascontiguousarray(embed_weight, dtype=np.float32),
        "convwT": convwT.astype(np.float32),
        "convb": conv_b.reshape(F, 1).astype(np.float32),
        "qwT": qwT,
        "outwc": outwc,
        "outbc": outbc.astype(np.float32),
    }


def make_in_maps(cfg, text, embed_weight, conv_w, conv_b, Q_w, out_w, out_b):
    shared = None
    in_maps = []
    for b in range(cfg["B"]):
        m = _prep_inputs(cfg, text[b], embed_weight, conv_w, conv_b,
                         Q_w, out_w, out_b)
        if shared is None:
            shared = {k: v for k, v in m.items() if k != "text"}
        else:
            for k, v in shared.items():
                m[k] = v
        in_maps.append(m)
    return in_maps


def assemble_outputs(cfg, results):
    B, C, T_OUT, TI, T2, CJ = (cfg["B"], cfg["C"], cfg["T_OUT"], cfg["TI"],
                               cfg["T2"], cfg["CJ"])
    logits = np.empty((B, C), np.float32)
    alpha = np.empty((B, C, T_OUT), np.float32)
    for b in range(B):
        lg = results[b]["logits"]              # [128, CJ], c = 128*j + p
        logits[b] = lg.T.reshape(CJ * 128)[:C]
        at = results[b]["alphaT"]              # [128, TI, C], t = 128*ti + p
        alpha[b] = at.transpose(2, 1, 0).reshape(C, T2)[:, :T_OUT]
    return logits, alpha


_CACHE = {}


def _get_kernel(reps=1):
    key = reps
    if key not in _CACHE:
        _CACHE[key] = build_kernel(reps=reps)
    return _CACHE[key]


def kernel(text, embed_weight, conv_w, conv_b, Q_w, out_w, out_b):
    text = np.asarray(text)
    nc, cfg = _get_kernel()
    in_maps = make_in_maps(cfg, text, np.asarray(embed_weight),
                           np.asarray(conv_w), np.asarray(conv_b),
                           np.asarray(Q_w), np.asarray(out_w),
                           np.asarray(out_b))
    res = run_bass_kernel_spmd(nc, in_maps, core_ids=list(range(N_CORES)))
    return assemble_outputs(cfg, res.results)


# revision 24
# speedup vs baseline: 2.6608x; 2.6608x over previous
"""Trainium2 Bass kernel for CAML-style sparse attention (nn_CAML_39977555591512).

Model (per batch element b):
    x   = embed[text[b]]                      # (L, E)
    Z   = tanh(conv1d(x, conv_w) + conv_b)    # (T, F), T = L + 1
    S   = Z @ Q_w.T                           # (T, C) scores (transposed layout)
    a   = softmax(S, axis=T)                  # alpha^T
    E   = a^T @ Z                             # (C, F)
    y   = sum_f out_w * E + out_b             # (C,)
Returns (logits (B, C), alpha (B, C, T)).

Sharding: pure data-parallel over batch (B = 8 = n_cores); every core runs the
identical single-core program on its own batch row.  All device work is done in
the [t-partition, c-free] orientation; alpha is written transposed (plus helper
padding) and the host restores the reference layout while gathering.
"""

import os
import numpy as np
from contextlib import ExitStack

os.environ.setdefault("MYCRO_LOCAL_CACHE", "1")

import concourse.bass as bass
import concourse.tile as tile
from concourse import bacc, mybir
from concourse.bass_utils import run_bass_kernel_spmd
from concourse.masks import make_identity

F32 = mybir.dt.float32
BF16 = mybir.dt.bfloat16
I32 = mybir.dt.int32

N_CORES = 8


def _cfg(B=8, L=2500, V=50000, E=100, F=50, K=10, C=8921):
    PAD = K // 2
    T_OUT = L + 2 * PAD - K + 1          # conv output length (2501)
    TI = (T_OUT + 127) // 128            # t-tiles of 128 (20)
    T2 = TI * 128                        # padded length (2560)
    T_TAIL = T_OUT - (TI - 1) * 128      # valid rows in last t-tile (69)
    CJ = (C + 127) // 128                # 128-wide class tiles (70)
    C2 = CJ * 128                        # padded classes (8960)
    XW = T2 + K - 1                      # padded x columns (2569)
    chunks = []
    c0 = 0
    while c0 < C2:
        chunks.append((c0, min(512, C2 - c0)))
        c0 += 512
    return dict(B=B, L=L, V=V, E=E, F=F, K=K, C=C, PAD=PAD, T_OUT=T_OUT, TI=TI,
                T2=T2, T_TAIL=T_TAIL, CJ=CJ, C2=C2, XW=XW, chunks=chunks)


def build_kernel(cfg=None, reps=1):
    """Build + compile the single-core Bacc program (run SPMD on 8 cores)."""
    cfg = cfg or _cfg()
    V, E, F, K = cfg["V"], cfg["E"], cfg["F"], cfg["K"]
    TI, T2, T_TAIL = cfg["TI"], cfg["T2"], cfg["T_TAIL"]
    CJ, C2, XW = cfg["CJ"], cfg["C2"], cfg["XW"]
    chunks = cfg["chunks"]

    nc = bacc.Bacc("TRN2", target_bir_lowering=False, debug=False,
                   num_devices=N_CORES)

    # ---- DRAM I/O (host pre-packs layouts; see _prep_inputs) ----
    text_ap = nc.dram_tensor("text", [128, TI], I32, kind="ExternalInput").ap()
    embed_ap = nc.dram_tensor("embed", [V, E], F32, kind="ExternalInput").ap()
    convwT_ap = nc.dram_tensor("convwT", [E, K * F], BF16, kind="ExternalInput").ap()
    convb_ap = nc.dram_tensor("convb", [F, 1], F32, kind="ExternalInput").ap()
    qwT_ap = nc.dram_tensor("qwT", [F, C2], BF16, kind="ExternalInput").ap()
    outw_ap = nc.dram_tensor("outwc", [128, CJ * F], F32, kind="ExternalInput").ap()
    outb_ap = nc.dram_tensor("outbc", [128, CJ], F32, kind="ExternalInput").ap()
    alphaT_ap = nc.dram_tensor("alphaT", [128, TI, cfg["C"]], F32,
                               kind="ExternalOutput").ap()
    logits_ap = nc.dram_tensor("logits", [128, CJ], F32, kind="ExternalOutput").ap()

    with tile.TileContext(nc) as tc, ExitStack() as ctx, \
            nc.allow_low_precision(reason="bf16 softmax/attention path; "
                                          "validated ~3e-3 rel err"):
        consts = ctx.enter_context(tc.tile_pool(name="consts", bufs=1))
        persist = ctx.enter_context(tc.tile_pool(name="persist", bufs=1))
        work = ctx.enter_context(tc.tile_pool(name="work", bufs=3))
        exps_pool = ctx.enter_context(tc.tile_pool(name="exps", bufs=2))
        alpha_pool = ctx.enter_context(tc.tile_pool(name="alpha", bufs=2))
        ps_scores = ctx.enter_context(tc.tile_pool(name="ps_scores", bufs=2, space="PSUM"))
        ps_ehat = ctx.enter_context(tc.tile_pool(name="ps_ehat", bufs=2, space="PSUM"))
        ps_misc = ctx.enter_context(tc.tile_pool(name="ps_misc", bufs=2, space="PSUM"))

        # ---- constants ----
        identity = consts.tile([128, 128], F32)
        make_identity(nc, identity[:])
        ident_bf = consts.tile([128, 128], BF16)
        nc.vector.tensor_copy(ident_bf[:], identity[:])
        ones_row = consts.tile([1, 128], BF16)
        nc.vector.memset(ones_row[:], 1.0)

        txt = consts.tile([128, TI], I32)
        nc.sync.dma_start(txt[:], text_ap[:])
        convwT = consts.tile([E, K * F], BF16)
        nc.sync.dma_start(convwT[:], convwT_ap[:])
        convb = consts.tile([F, 1], F32)
        nc.sync.dma_start(convb[:], convb_ap[:])
        qwT = consts.tile([F, C2], BF16)
        nc.sync.dma_start(qwT[:], qwT_ap[:])
        outw = consts.tile([128, CJ * F], F32)
        nc.sync.dma_start(outw[:], outw_ap[:])
        outb = consts.tile([128, CJ], F32)
        nc.sync.dma_start(outb[:], outb_ap[:])

        for _rep in range(reps):
            # ---- embedding gather -> xT [E, XW] (x_pad transposed, bf16) ----
            xT = persist.tile([E, XW], BF16)
            nc.vector.memset(xT[:], 0.0)
            for g in range(TI):
                gt = work.tile([128, E], F32, tag="gather")
                nc.gpsimd.indirect_dma_start(
                    out=gt[:], out_offset=None, in_=embed_ap[:],
                    in_offset=bass.IndirectOffsetOnAxis(ap=txt[:, g:g + 1], axis=0),
                )
                xp = ps_misc.tile([E, 128], F32, tag="misc")
                nc.tensor.transpose(xp[:], gt[:], identity[:])
                nc.vector.tensor_copy(xT[:, cfg["PAD"] + 128 * g:
                                         cfg["PAD"] + 128 * g + 128], xp[:])

            # ---- conv1d + tanh -> ZT [F, T2] (bf16) ----
            ZT = persist.tile([F, T2], BF16)
            for t0 in range(0, T2, 512):
                w = min(512, T2 - t0)
                cp = ps_misc.tile([F, 512], F32, tag="misc")
                for k in range(K):
                    nc.tensor.matmul(cp[:, :w], lhsT=convwT[:, k * F:(k + 1) * F],
                                     rhs=xT[:, k + t0:k + t0 + w],
                                     start=(k == 0), stop=(k == K - 1))
                nc.scalar.activation(ZT[:, t0:t0 + w], cp[:, :w],
                                     mybir.ActivationFunctionType.Tanh,
                                     bias=convb[:], scale=1.0)

            # ---- Z_aug [128, 1+F] per t-tile (bf16; col 0 = ones; tail zeroed)
            # Layout puts the softmax-denominator row at PSUM partition 0
            # (DVE PSUM reads need a 32-aligned base partition).
            zaug = persist.tile([128, TI * (F + 1)], BF16)
            nc.vector.memset(zaug[:], 0.0)
            for ti in range(TI):
                zp = ps_misc.tile([128, F], BF16, tag="misc")
                nc.tensor.transpose(zp[:], ZT[:, 128 * ti:128 * ti + 128],
                                    ident_bf[:F, :F])
                rows = 128 if ti < TI - 1 else T_TAIL
                nc.vector.memset(zaug[:rows, ti * (F + 1):ti * (F + 1) + 1], 1.0)
                nc.vector.tensor_copy(zaug[:rows, ti * (F + 1) + 1:
                                           (ti + 1) * (F + 1)],
                                      zp[:rows, :])

            # ---- logits accumulators (class-partition layout) ----
            reds = persist.tile([128, CJ], F32, tag="reds")
            rcols = persist.tile([128, CJ], F32, tag="rcols")

            # ---- main loop over class chunks ----
            for (c0, NC) in chunks:
                ncr = min(NC, cfg["C"] - c0)  # columns actually stored
                exps = exps_pool.tile([128, TI * NC], BF16, tag="exps")
                # phase A: scores^T -> exp -> E_hat accumulation
                eh = ps_ehat.tile([F + 1, NC], F32, tag="ehat")
                for ti0 in range(0, TI, 2):
                    tis = [ti for ti in (ti0, ti0 + 1) if ti < TI]
                    sp = ps_scores.tile([128, 1024], F32, tag="scores")
                    for h, ti in enumerate(tis):
                        nc.tensor.matmul(sp[:, 512 * h:512 * h + NC],
                                         lhsT=ZT[:, 128 * ti:128 * ti + 128],
                                         rhs=qwT[:, c0:c0 + NC],
                                         start=True, stop=True)
                    if NC == 512 and len(tis) == 2:
                        nc.scalar.activation(
                            exps[:, ti0 * NC:(ti0 + 2) * NC],
                            sp[:], mybir.ActivationFunctionType.Exp)
                    else:
                        for h, ti in enumerate(tis):
                            nc.scalar.activation(
                                exps[:, ti * NC:(ti + 1) * NC],
                                sp[:, 512 * h:512 * h + NC],
                                mybir.ActivationFunctionType.Exp)
                    for ti in tis:
                        nc.tensor.matmul(
                            eh[:], lhsT=zaug[:, ti * (F + 1):(ti + 1) * (F + 1)],
                            rhs=exps[:, ti * NC:(ti + 1) * NC],
                            start=(ti == 0), stop=(ti == TI - 1))

                # phase B: normalize + store alpha, logits partials
                r_row = work.tile([1, NC], BF16, tag="rrow")
                nc.vector.reciprocal(r_row[:], eh[0:1, :])
                rp = ps_misc.tile([128, NC], F32, tag="misc")
                nc.tensor.matmul(rp[:], lhsT=ones_row[:], rhs=r_row[:],
                                 start=True, stop=True)
                rb = work.tile([128, NC], BF16, tag="rb")
                nc.vector.tensor_copy(rb[:], rp[:])

                ehs = work.tile([F + 1, NC], F32, tag="ehs")
                nc.vector.tensor_copy(ehs[:], eh[:])

                alpha = alpha_pool.tile([128, TI * NC], BF16, tag="alpha")
                for ti in range(TI):
                    nc.vector.tensor_mul(alpha[:, ti * NC:(ti + 1) * NC],
                                         exps[:, ti * NC:(ti + 1) * NC], rb[:])
                nc.gpsimd.dma_start(
                    out=alphaT_ap[:, :, c0:c0 + ncr],
                    in_=alpha[:].rearrange("p (ti c) -> p ti c", ti=TI)[:, :, :ncr])

                for jj in range(NC // 128):
                    j = c0 // 128 + jj
                    et = ps_misc.tile([128, F + 1], F32, tag="misc")
                    nc.tensor.transpose(et[:], ehs[:, 128 * jj:128 * jj + 128],
                                        identity[:F + 1, :F + 1])
                    nc.vector.reciprocal(rcols[:, j:j + 1], et[:, 0:1])
                    pr = work.tile([128, F], F32, tag="prod")
                    nc.vector.tensor_mul(pr[:], et[:, 1:F + 1],
                                         outw[:, j * F:(j + 1) * F])
                    nc.vector.tensor_reduce(reds[:, j:j + 1], pr[:],
                                            axis=mybir.AxisListType.X,
                                            op=mybir.AluOpType.add)

            # ---- logits = reds * rcols + outb ----
            lg = persist.tile([128, CJ], F32, tag="lg")
            nc.vector.tensor_mul(lg[:], reds[:], rcols[:])
            nc.vector.tensor_add(lg[:], lg[:], outb[:])
            nc.sync.dma_start(logits_ap[:], lg[:])

    nc.compile()
    return nc, cfg


def _prep_inputs(cfg, text, embed_weight, conv_w, conv_b, Q_w, out_w, out_b):
    """Host-side packing of one batch row's inputs into the kernel layouts."""
    TI, T2, CJ, C2 = cfg["TI"], cfg["T2"], cfg["CJ"], cfg["C2"]
    E, F, K, C, L = cfg["E"], cfg["F"], cfg["K"], cfg["C"], cfg["L"]

    tp = np.zeros(T2, np.int32)
    tp[:L] = text
    txt2d = np.ascontiguousarray(tp.reshape(TI, 128).T)

    import ml_dtypes
    # slice k of convwT[:, k*F:(k+1)*F] must be conv_w[:, :, k].T = [E, F]
    convwT = np.ascontiguousarray(
        conv_w.transpose(1, 2, 0).reshape(E, K * F)).astype(ml_dtypes.bfloat16)

    qwT = np.zeros((F, C2), ml_dtypes.bfloat16)
    qwT[:, :C] = Q_w.T.astype(ml_dtypes.bfloat16)

    outw_pad = np.zeros((C2, F), np.float32)
    outw_pad[:C] = out_w
    outwc = np.ascontiguousarray(
        outw_pad.reshape(CJ, 128, F).transpose(1, 0, 2).reshape(128, CJ * F))

    outb_pad = np.zeros(C2, np.float32)
    outb_pad[:C] = out_b
    outbc = np.ascontiguousarray(outb_pad.reshape(CJ, 128).T)

    return {
        "text": txt2d,
        "embed": np.ascontiguousarray(embed_weight, dtype=np.float32),
        "convwT": convwT.astype(np.float32),
        "convb": conv_b.reshape(F, 1).astype(np.float32),
        "qwT": qwT,
        "outwc": outwc,
        "outbc": outbc.astype(np.float32),
    }


def make_in_maps(cfg, text, embed_weight, conv_w, conv_b, Q_w, out_w, out_b):
    shared = None
    in_maps = []
    for b in range(cfg["B"]):
        m = _prep_inputs(cfg, text[b], embed_weight, conv_w, conv_b,
                         Q_w, out_w, out_b)
        if shared is None:
            shared = {k: v for k, v in m.items() if k != "text"}
        else:
            for k, v in shared.items():
                m[k] = v
        in_maps.append(m)
    return in_maps


def assemble_outputs(cfg, results):
    B, C, T_OUT, TI, T2, CJ = (cfg["B"], cfg["C"], cfg["T_OUT"], cfg["TI"],
                               cfg["T2"], cfg["CJ"])
    logits = np.empty((B, C), np.float32)
    alpha = np.empty((B, C, T_OUT), np.float32)
    for b in range(B):
        lg = results[b]["logits"]              # [128, CJ], c = 128*j + p
        logits[b] = lg.T.reshape(CJ * 128)[:C]
        at = results[b]["alphaT"]              # [128, TI, C], t = 128*ti + p
        alpha[b] = at.transpose(2, 1, 0).reshape(C, T2)[:, :T_OUT]
    return logits, alpha


_CACHE = {}


def _get_kernel(reps=1):
    key = reps
    if key not in _CACHE:
        _CACHE[key] = build_kernel(reps=reps)
    return _CACHE[key]


def kernel(text, embed_weight, conv_w, conv_b, Q_w, out_w, out_b):
    text = np.asarray(text)
    nc, cfg = _get_kernel()
    in_maps = make_in_maps(cfg, text, np.asarray(embed_weight),
                           np.asarray(conv_w), np.asarray(conv_b),
                           np.asarray(Q_w), np.asarray(out_w),
                           np.asarray(out_b))
    res = run_bass_kernel_spmd(nc, in_maps, core_ids=list(range(N_CORES)))
    return assemble_outputs(cfg, res.results)


# revision 29
# speedup vs baseline: 4.6528x; 1.7487x over previous
"""Trainium2 Bass kernel for CAML-style sparse attention (nn_CAML_39977555591512).

Model (per batch element b):
    x   = embed[text[b]]                      # (L, E)
    Z   = tanh(conv1d(x, conv_w) + conv_b)    # (T, F), T = L + 1
    S   = Z @ Q_w.T                           # (T, C) scores (transposed layout)
    a   = softmax(S, axis=T)                  # alpha^T
    E   = a^T @ Z                             # (C, F)
    y   = sum_f out_w * E + out_b             # (C,)
Returns (logits (B, C), alpha (B, C, T)).

Sharding: pure data-parallel over batch (B = 8 = n_cores); every core runs the
identical single-core program on its own batch row.  All device work is done in
the [t-partition, c-free] orientation; alpha is written transposed (plus helper
padding) and the host restores the reference layout while gathering.
"""

import os
import numpy as np
from contextlib import ExitStack

os.environ.setdefault("MYCRO_LOCAL_CACHE", "1")

import concourse.bass as bass
import concourse.tile as tile
from concourse import bacc, mybir
from concourse.bass_utils import run_bass_kernel_spmd
from concourse.masks import make_identity

F32 = mybir.dt.float32
BF16 = mybir.dt.bfloat16
I32 = mybir.dt.int32

N_CORES = 8


def _cfg(B=8, L=2500, V=50000, E=100, F=50, K=10, C=8921):
    PAD = K // 2
    T_OUT = L + 2 * PAD - K + 1          # conv output length (2501)
    TI = (T_OUT + 127) // 128            # t-tiles of 128 (20)
    T2 = TI * 128                        # padded length (2560)
    T_TAIL = T_OUT - (TI - 1) * 128      # valid rows in last t-tile (69)
    CJ = (C + 127) // 128                # 128-wide class tiles (70)
    C2 = CJ * 128                        # padded classes (8960)
    XW = T2 + K - 1                      # padded x columns (2569)
    chunks = []
    c0 = 0
    while c0 < C2:
        chunks.append((c0, min(512, C2 - c0)))
        c0 += 512
    return dict(B=B, L=L, V=V, E=E, F=F, K=K, C=C, PAD=PAD, T_OUT=T_OUT, TI=TI,
                T2=T2, T_TAIL=T_TAIL, CJ=CJ, C2=C2, XW=XW, chunks=chunks)


def build_kernel(cfg=None, reps=1, variant=""):
    """Build + compile the single-core Bacc program (run SPMD on 8 cores).

    variant: "" (normal) | "noalpha" (skip alpha store; timing ablation) |
             "bf16out" (alpha stored as bf16; host upcasts) |
             "nogather" (direct DMA instead of indirect gather; ablation)
    """
    cfg = cfg or _cfg()
    V, E, F, K = cfg["V"], cfg["E"], cfg["F"], cfg["K"]
    TI, T2, T_TAIL = cfg["TI"], cfg["T2"], cfg["T_TAIL"]
    CJ, C2, XW = cfg["CJ"], cfg["C2"], cfg["XW"]
    chunks = cfg["chunks"]

    nc = bacc.Bacc("TRN2", target_bir_lowering=False, debug=False,
                   num_devices=N_CORES)

    # ---- DRAM I/O (host pre-packs layouts; see _prep_inputs) ----
    text_ap = nc.dram_tensor("text", [128, TI], I32, kind="ExternalInput").ap()
    embed_ap = nc.dram_tensor("embed", [V, E], F32, kind="ExternalInput").ap()
    convwT_ap = nc.dram_tensor("convwT", [E, K * F], BF16, kind="ExternalInput").ap()
    convb_ap = nc.dram_tensor("convb", [F, 1], F32, kind="ExternalInput").ap()
    qwT_ap = nc.dram_tensor("qwT", [F, C2], BF16, kind="ExternalInput").ap()
    outw_ap = nc.dram_tensor("outwc", [128, CJ * F], F32, kind="ExternalInput").ap()
    outb_ap = nc.dram_tensor("outbc", [128, CJ], F32, kind="ExternalInput").ap()
    alphaT_ap = nc.dram_tensor("alphaT", [128, TI, cfg["C"]],
                               BF16 if variant == "bf16out" else F32,
                               kind="ExternalOutput").ap()
    logits_ap = nc.dram_tensor("logits", [128, CJ], F32, kind="ExternalOutput").ap()

    with tile.TileContext(nc) as tc, ExitStack() as ctx, \
            nc.allow_low_precision(reason="bf16 softmax/attention path; "
                                          "validated ~3e-3 rel err"):
        consts = ctx.enter_context(tc.tile_pool(name="consts", bufs=1))
        persist = ctx.enter_context(tc.tile_pool(name="persist", bufs=1))
        work = ctx.enter_context(tc.tile_pool(name="work", bufs=3))
        exps_pool = ctx.enter_context(tc.tile_pool(name="exps", bufs=2))
        alpha_pool = ctx.enter_context(tc.tile_pool(name="alpha", bufs=2))
        ps_scores = ctx.enter_context(tc.tile_pool(name="ps_scores", bufs=2, space="PSUM"))
        ps_ehat = ctx.enter_context(tc.tile_pool(name="ps_ehat", bufs=2, space="PSUM"))
        ps_misc = ctx.enter_context(tc.tile_pool(name="ps_misc", bufs=2, space="PSUM"))

        # ---- constants ----
        identity = consts.tile([128, 128], F32)
        make_identity(nc, identity[:])
        ident_bf = consts.tile([128, 128], BF16)
        nc.vector.tensor_copy(ident_bf[:], identity[:])
        ones_row = consts.tile([1, 128], BF16)
        nc.vector.memset(ones_row[:], 1.0)

        txt = consts.tile([128, TI], I32)
        nc.sync.dma_start(txt[:], text_ap[:])
        convwT = consts.tile([E, K * F], BF16)
        nc.sync.dma_start(convwT[:], convwT_ap[:])
        convb = consts.tile([F, 1], F32)
        nc.sync.dma_start(convb[:], convb_ap[:])
        qwT = consts.tile([F, C2], BF16)
        nc.sync.dma_start(qwT[:], qwT_ap[:])
        outw = consts.tile([128, CJ * F], F32)
        nc.sync.dma_start(outw[:], outw_ap[:])
        outb = consts.tile([128, CJ], F32)
        nc.sync.dma_start(outb[:], outb_ap[:])

        for _rep in range(reps):
            # ---- embedding gather -> xT [E, XW] (x_pad transposed, bf16) ----
            xT = persist.tile([E, XW], BF16)
            nc.vector.memset(xT[:], 0.0)
            for g in range(TI):
                gt = work.tile([128, E], F32, tag="gather")
                if variant == "nogather":
                    nc.sync.dma_start(gt[:], embed_ap[128 * g:128 * g + 128, :])
                else:
                    nc.gpsimd.indirect_dma_start(
                        out=gt[:], out_offset=None, in_=embed_ap[:],
                        in_offset=bass.IndirectOffsetOnAxis(ap=txt[:, g:g + 1],
                                                            axis=0),
                    )
                xp = ps_misc.tile([E, 128], F32, tag="misc")
                nc.tensor.transpose(xp[:], gt[:], identity[:])
                nc.vector.tensor_copy(xT[:, cfg["PAD"] + 128 * g:
                                         cfg["PAD"] + 128 * g + 128], xp[:])

            # ---- conv1d + tanh -> ZT [F, T2] (bf16) ----
            ZT = persist.tile([F, T2], BF16)
            for t0 in range(0, T2, 512):
                w = min(512, T2 - t0)
                cp = ps_misc.tile([F, 512], F32, tag="misc")
                for k in range(K):
                    nc.tensor.matmul(cp[:, :w], lhsT=convwT[:, k * F:(k + 1) * F],
                                     rhs=xT[:, k + t0:k + t0 + w],
                                     start=(k == 0), stop=(k == K - 1))
                nc.scalar.activation(ZT[:, t0:t0 + w], cp[:, :w],
                                     mybir.ActivationFunctionType.Tanh,
                                     bias=convb[:], scale=1.0)

            # ---- Z_aug [128, 1+F] per t-tile (bf16; col 0 = ones; tail zeroed)
            # Layout puts the softmax-denominator row at PSUM partition 0
            # (DVE PSUM reads need a 32-aligned base partition).
            zaug = persist.tile([128, TI * (F + 1)], BF16)
            nc.vector.memset(zaug[:], 0.0)
            for ti in range(TI):
                zp = ps_misc.tile([128, F], BF16, tag="misc")
                nc.tensor.transpose(zp[:], ZT[:, 128 * ti:128 * ti + 128],
                                    ident_bf[:F, :F])
                rows = 128 if ti < TI - 1 else T_TAIL
                nc.vector.memset(zaug[:rows, ti * (F + 1):ti * (F + 1) + 1], 1.0)
                nc.vector.tensor_copy(zaug[:rows, ti * (F + 1) + 1:
                                           (ti + 1) * (F + 1)],
                                      zp[:rows, :])

            # ---- logits accumulators (class-partition layout) ----
            reds = persist.tile([128, CJ], F32, tag="reds")
            rcols = persist.tile([128, CJ], F32, tag="rcols")

            # ---- main loop over class chunks ----
            for (c0, NC) in chunks:
                ncr = min(NC, cfg["C"] - c0)  # columns actually stored
                exps = exps_pool.tile([128, TI * NC], BF16, tag="exps")
                # phase A: scores^T -> exp -> E_hat accumulation
                eh = ps_ehat.tile([F + 1, NC], F32, tag="ehat")
                for ti0 in range(0, TI, 2):
                    tis = [ti for ti in (ti0, ti0 + 1) if ti < TI]
                    sp = ps_scores.tile([128, 1024], F32, tag="scores")
                    for h, ti in enumerate(tis):
                        nc.tensor.matmul(sp[:, 512 * h:512 * h + NC],
                                         lhsT=ZT[:, 128 * ti:128 * ti + 128],
                                         rhs=qwT[:, c0:c0 + NC],
                                         start=True, stop=True)
                    if NC == 512 and len(tis) == 2:
                        nc.scalar.activation(
                            exps[:, ti0 * NC:(ti0 + 2) * NC],
                            sp[:], mybir.ActivationFunctionType.Exp)
                    else:
                        for h, ti in enumerate(tis):
                            nc.scalar.activation(
                                exps[:, ti * NC:(ti + 1) * NC],
                                sp[:, 512 * h:512 * h + NC],
                                mybir.ActivationFunctionType.Exp)
                    for ti in tis:
                        nc.tensor.matmul(
                            eh[:], lhsT=zaug[:, ti * (F + 1):(ti + 1) * (F + 1)],
                            rhs=exps[:, ti * NC:(ti + 1) * NC],
                            start=(ti == 0), stop=(ti == TI - 1))

                # phase B: normalize + store alpha, logits partials
                r_row = work.tile([1, NC], BF16, tag="rrow")
                nc.vector.reciprocal(r_row[:], eh[0:1, :])
                rp = ps_misc.tile([128, NC], F32, tag="misc")
                nc.tensor.matmul(rp[:], lhsT=ones_row[:], rhs=r_row[:],
                                 start=True, stop=True)
                rb = work.tile([128, NC], BF16, tag="rb")
                nc.vector.tensor_copy(rb[:], rp[:])

                ehs = work.tile([F + 1, NC], F32, tag="ehs")
                nc.vector.tensor_copy(ehs[:], eh[:])

                alpha = alpha_pool.tile([128, TI * NC], BF16, tag="alpha")
                for ti in range(TI):
                    nc.vector.tensor_mul(alpha[:, ti * NC:(ti + 1) * NC],
                                         exps[:, ti * NC:(ti + 1) * NC], rb[:])
                if variant != "noalpha":
                    alpha_view = alpha[:].rearrange("p (ti c) -> p ti c",
                                                    ti=TI)[:, :, :ncr]
                    if variant == "bf16out":
                        nc.sync.dma_start(out=alphaT_ap[:, :, c0:c0 + ncr],
                                          in_=alpha_view)
                    else:
                        nc.gpsimd.dma_start(out=alphaT_ap[:, :, c0:c0 + ncr],
                                            in_=alpha_view)

                for jj in range(NC // 128):
                    j = c0 // 128 + jj
                    et = ps_misc.tile([128, F + 1], F32, tag="misc")
                    nc.tensor.transpose(et[:], ehs[:, 128 * jj:128 * jj + 128],
                                        identity[:F + 1, :F + 1])
                    nc.vector.reciprocal(rcols[:, j:j + 1], et[:, 0:1])
                    pr = work.tile([128, F], F32, tag="prod")
                    nc.vector.tensor_mul(pr[:], et[:, 1:F + 1],
                                         outw[:, j * F:(j + 1) * F])
                    nc.vector.tensor_reduce(reds[:, j:j + 1], pr[:],
                                            axis=mybir.AxisListType.X,
                                            op=mybir.AluOpType.add)

            # ---- logits = reds * rcols + outb ----
            lg = persist.tile([128, CJ], F32, tag="lg")
            nc.vector.tensor_mul(lg[:], reds[:], rcols[:])
            nc.vector.tensor_add(lg[:], lg[:], outb[:])
            nc.sync.dma_start(logits_ap[:], lg[:])

    nc.compile()
    return nc, cfg


def _prep_inputs(cfg, text, embed_weight, conv_w, conv_b, Q_w, out_w, out_b):
    """Host-side packing of one batch row's inputs into the kernel layouts."""
    TI, T2, CJ, C2 = cfg["TI"], cfg["T2"], cfg["CJ"], cfg["C2"]
    E, F, K, C, L = cfg["E"], cfg["F"], cfg["K"], cfg["C"], cfg["L"]

    tp = np.zeros(T2, np.int32)
    tp[:L] = text
    txt2d = np.ascontiguousarray(tp.reshape(TI, 128).T)

    import ml_dtypes
    # slice k of convwT[:, k*F:(k+1)*F] must be conv_w[:, :, k].T = [E, F]
    convwT = np.ascontiguousarray(
        conv_w.transpose(1, 2, 0).reshape(E, K * F)).astype(ml_dtypes.bfloat16)

    qwT = np.zeros((F, C2), ml_dtypes.bfloat16)
    qwT[:, :C] = Q_w.T.astype(ml_dtypes.bfloat16)

    outw_pad = np.zeros((C2, F), np.float32)
    outw_pad[:C] = out_w
    outwc = np.ascontiguousarray(
        outw_pad.reshape(CJ, 128, F).transpose(1, 0, 2).reshape(128, CJ * F))

    outb_pad = np.zeros(C2, np.float32)
    outb_pad[:C] = out_b
    outbc = np.ascontiguousarray(outb_pad.reshape(CJ, 128).T)

    return {
        "text": txt2d,
        "embed": np.ascontiguousarray(embed_weight, dtype=np.float32),
        "convwT": convwT.astype(np.float32),
        "convb": conv_b.reshape(F, 1).astype(np.float32),
        "qwT": qwT,
        "outwc": outwc,
        "outbc": outbc.astype(np.float32),
    }


def make_in_maps(cfg, text, embed_weight, conv_w, conv_b, Q_w, out_w, out_b):
    shared = None
    in_maps = []
    for b in range(cfg["B"]):
        m = _prep_inputs(cfg, text[b], embed_weight, conv_w, conv_b,
                         Q_w, out_w, out_b)
        if shared is None:
            shared = {k: v for k, v in m.items() if k != "text"}
        else:
            for k, v in shared.items():
                m[k] = v
        in_maps.append(m)
    return in_maps


def assemble_outputs(cfg, results):
    B, C, T_OUT, TI, T2, CJ = (cfg["B"], cfg["C"], cfg["T_OUT"], cfg["TI"],
                               cfg["T2"], cfg["CJ"])
    logits = np.empty((B, C), np.float32)
    alpha = np.empty((B, C, T_OUT), np.float32)
    for b in range(B):
        lg = results[b]["logits"]              # [128, CJ], c = 128*j + p
        logits[b] = lg.T.reshape(CJ * 128)[:C]
        at = results[b]["alphaT"]              # [128, TI, C], t = 128*ti + p
        alpha[b] = at.transpose(2, 1, 0).reshape(C, T2)[:, :T_OUT].astype(
            np.float32)
    return logits, alpha


_CACHE = {}


def _get_kernel(reps=1):
    key = reps
    if key not in _CACHE:
        _CACHE[key] = build_kernel(reps=reps)
    return _CACHE[key]


def kernel(text, embed_weight, conv_w, conv_b, Q_w, out_w, out_b):
    text = np.asarray(text)
    nc, cfg = _get_kernel()
    in_maps = make_in_maps(cfg, text, np.asarray(embed_weight),
                           np.asarray(conv_w), np.asarray(conv_b),
                           np.asarray(Q_w), np.asarray(out_w),
                           np.asarray(out_b))
    res = run_bass_kernel_spmd(nc, in_maps, core_ids=list(range(N_CORES)))
    return assemble_outputs(cfg, res.results)
